# revision 43
# baseline (speedup 1.0000x reference)
# Trainium2 Bass kernel for masked causal attention
#   B=2, H=16, S=2048, D=64, bool attn_mask [B, S, S] + causal, softmax, @V.
#
# Sharding: 8 cores x 4 heads (cores 0-3 -> batch 0, cores 4-7 -> batch 1).
# Each core computes its 4 heads fully on-device; the per-batch mask is
# resident in SBUF and shared by the core's 4 heads.
#
# QK runs ROW-TILED: contraction is d=64, so the 128x128 PE array is split
# into two 64x128 tiles (T0 rows 0-63, T8 rows 64-127) that compute TWO
# k-tiles (a "pair") CONCURRENTLY -- q^T is duplicated on both partition
# halves, K^T of even k-tiles lives on partitions 0-63 and odd k-tiles on
# 64-127.  This halves QK stream time vs the old zero-padded 128-row form.
# PV stays full 128x128 (contraction = 128 keys); QK<->PV transitions now
# cost a tiling-mode drain (~140ns), so PVs are drained in batches.
#
# Per (head, pair i of k-tiles (2i, 2i+1), 512-wide q-segment from 256*i):
#   S^T[k, q] even/odd = concurrent 64-row matmuls -> [128, 2, 512] PSUM
#   softmax tile (one op over both halves):
#     2/3 of segs: ACT exp(s * ln2/1024) then DVE mask-multiply
#     1/3 of segs: ONE DVE scalar_tensor_tensor: i16 = (s*C + B)*mask,
#                  p = bitcast_fp16(i16)   (Q pre-scaled by C on host)
#   outT[m,q] += sum_k vp[k,m] p[k,q]   (PE: lhsT=[V | ones] -> row 64 = denom)
# The normalization (num/den) happens ON HOST: the kernel ships the
# unnormalized [65, S] accumulator as fp16.
# Causal structure: pair i only computes q >= 256*i; the odd tile's
# below-diagonal 128-col block is masked in softmax and skipped in PV.

import os
import numpy as np

B, H, S, D = 2, 16, 2048, 64
NCORES = 8
HPC = 4          # heads per core
P = 128
NKT = S // P     # 16 k-tiles
NPAIR = NKT // 2
CHUNK = 1024     # st psum tile: [128, 2, 512] f32 = 2 PSUM banks
SCHRAUD_IDX = {1}
SCHRAUD_MOD = 3
PV_DELAY = int(os.environ.get("ATTN_PV_DELAY", "8"))
PV_POP = int(os.environ.get("ATTN_PV_POP", "10"))
# warm-up sized so it ends right as the (u8-shrunk) head-0 critical DMA
# set lands; enough sustained MMs to promote the HAM clock gate (~3.4us).
WARM_MM = int(os.environ.get("ATTN_WARM_MM", "8"))

# Schraudolph constants for p = exp(s * 0.125) via fp16 bit pattern:
#   i16 = int16(s*C_S + B_S);  p = bitcast_fp16(i16);  c=44 centers the
# relative error of the linear-mantissa approximation (max ~3.1%, rms ~2%).
C_SCH = float(0.125 * 1024.0 / np.log(2.0))
B_SCH = float(15.0 * 1024.0 - 44.0)
ACT_SCALE = float(np.log(2.0) / 1024.0)   # undoes the host C_SCH prescale

_cache = {}


def build_nc():
    import concourse.bacc as bacc
    import concourse.mybir as mybir
    import concourse.tile as tile
    from contextlib import ExitStack

    fp16 = mybir.dt.float16
    f32 = mybir.dt.float32
    i16 = mybir.dt.int16
    Exp = mybir.ActivationFunctionType.Exp
    Copy = mybir.ActivationFunctionType.Copy
    Mult = mybir.AluOpType.mult
    Add = mybir.AluOpType.add

    nc = bacc.Bacc("TRN2", target_bir_lowering=False, debug=False,
                   num_devices=NCORES)

    # Host-prepared, per-core inputs.  qt: q^T * C_SCH duplicated on both
    # partition halves (rows 0-63 == rows 64-127).  kt: K^T packed in pairs,
    # partitions 0-63 = even k-tiles, 64-127 = odd k-tiles.
    u8 = mybir.dt.uint8
    qt_d = nc.dram_tensor("qt", [HPC, P, S], fp16, kind="ExternalInput")
    kt_d = nc.dram_tensor("kt", [HPC, P, NPAIR, P], fp16, kind="ExternalInput")
    vp_d = nc.dram_tensor("vp", [HPC, P, NKT, D + 1], fp16, kind="ExternalInput")
    mk_d = nc.dram_tensor("maskt", [P, NKT, S], fp16, kind="ExternalInput")
    # uint8 copy of mask planes 0-1: halves the head-0 critical DMA bytes;
    # consumed only by head-0 pair-0's (forced) schraud path, whose STT op
    # runs at 1x regardless of the in1 dtype.
    mk8_d = nc.dram_tensor("mask8", [P, 2, S], u8, kind="ExternalInput")
    # unnormalized output: rows 0..63 = numerator^T, row 64 = denominator
    out_d = nc.dram_tensor("outt", [HPC, D + 1, S], fp16, kind="ExternalOutput")

    with tile.TileContext(nc) as tc, ExitStack() as ctx:
        mask_pool = ctx.enter_context(tc.tile_pool(name="mask", bufs=1))
        qk_pool = ctx.enter_context(tc.tile_pool(name="qk", bufs=2))
        vp_pool = ctx.enter_context(tc.tile_pool(name="vpool", bufs=2))
        p_pool = ctx.enter_context(tc.tile_pool(name="p", bufs=11))
        s_pool = ctx.enter_context(tc.tile_pool(name="sch", bufs=6))
        o_pool = ctx.enter_context(tc.tile_pool(name="osb", bufs=4))
        warm_pool = ctx.enter_context(tc.tile_pool(name="warm", bufs=1))
        st_psum = ctx.enter_context(tc.tile_pool(name="st", bufs=2, space="PSUM"))
        o_psum = ctx.enter_context(tc.tile_pool(name="outp", bufs=1, space="PSUM"))

        # PE warm-up on zeros: runs while the first input DMAs land and
        # opens the HAM clock gate to 2.4 GHz before the first real QK.
        wsb = warm_pool.tile([P, 512], fp16, tag="warm")
        nc.vector.memset(wsb[:], 0.0)
        wps = st_psum.tile([P, 2, 512], f32, tag="st")
        for i in range(WARM_MM):
            nc.tensor.matmul(wps[:, i % 2, :], lhsT=wsb[:, 0:P],
                             rhs=wsb[:], start=True, stop=True)

        def load_head(h):
            qt = qk_pool.tile([P, S], fp16, tag="qt")
            kt = qk_pool.tile([P, NPAIR, P], fp16, tag="kt")
            vp = vp_pool.tile([P, NKT, D + 1], fp16, tag="vp")
            nc.sync.dma_start(qt[:], qt_d[h])
            nc.sync.dma_start(kt[:], kt_d[h])
            nc.sync.dma_start(vp[:], vp_d[h])
            return qt, kt, vp

        # Start-latency-ordered first loads: the pair-0 seg-0 QK unblocks on
        # kt pair 0 (32KB) + first 512 q's (128KB), then mask planes 0-1 for
        # the first softmax, then the rest of head 0; remaining mask planes
        # stream behind.  Causal trim: k-tile g's mask is only read for
        # q >= 128*(g - g%2), so skip the lower-triangle bytes.
        qt0 = qk_pool.tile([P, S], fp16, tag="qt")
        kt0 = qk_pool.tile([P, NPAIR, P], fp16, tag="kt")
        vp0 = vp_pool.tile([P, NKT, D + 1], fp16, tag="vp")
        mask_sb = mask_pool.tile([P, NKT, S], fp16, tag="mask")
        # first-use order with coarse kicks (each dma_start costs ~0.65us
        # of Sync issue time).  Head 0 processes pair 7 FIRST: its deps
        # (kt pair 7 + qt/mask tails, ~224KB) land ~5us before pair 0's
        # ~1.9MB set, so the PE starts real work early.
        mk8_sb = mask_pool.tile([P, 2, S], u8, tag="mask8")
        nc.sync.dma_start(kt0[:, 0:1, :], kt_d[0, :, 0:1, :])
        nc.sync.dma_start(qt0[:, 0:512], qt_d[0, :, 0:512])
        nc.sync.dma_start(mk8_sb[:, :, 0:512], mk8_d[:, :, 0:512])
        nc.sync.dma_start(qt0[:, 512:CHUNK], qt_d[0, :, 512:CHUNK])
        nc.sync.dma_start(mk8_sb[:, :, 512:], mk8_d[:, :, 512:])
        nc.sync.dma_start(kt0[:, 1:, :], kt_d[0, :, 1:, :])
        nc.sync.dma_start(qt0[:, CHUNK:], qt_d[0, :, CHUNK:])
        nc.sync.dma_start(vp0[:], vp_d[0])
        head_tiles = {0: (qt0, kt0, vp0)}
        for g in range(2, NKT):
            c0 = P * (g - (g % 2))
            nc.sync.dma_start(mask_sb[:, g:g + 1, c0:], mk_d[:, g:g + 1, c0:])
        # fp16 planes 0-1 are only needed from head 1 onward (~40us in):
        # keep them off the head-0 critical path entirely.
        nc.sync.dma_start(mask_sb[:, 0:2, :], mk_d[:, 0:2, :])

        for h in range(HPC):
            qt, kt, vp = head_tiles.pop(h, None) or load_head(h)
            outp = o_psum.tile([D + 1, S], f32, tag="outp")
            pending_pv = []

            def emit_evac(b):
                # bank b of outp ([65, 512] f32) is fully accumulated ->
                # convert to fp16 in SBUF and ship; host divides num/den.
                s0, s1 = 512 * b, 512 * (b + 1)
                osb = o_pool.tile([D + 1, 512], fp16, tag="osb")
                # 2.5 evacs on ACT / 1.5 on DVE on average (alternating by
                # head): post-rebalance ACT ~64.3us vs DVE ~65.1us busy.
                # (Copy lives in the same ACT table as Exp: no table reload.)
                if b not in ((1,) if h % 2 == 0 else (1, 3)):
                    nc.scalar.activation(osb[:], outp[:, s0:s1], Copy)
                else:
                    nc.vector.tensor_copy(osb[:], outp[:, s0:s1])
                nc.sync.dma_start(out_d[h, :, s0:s1], osb[:])

            # Head 0 runs pair 7 FIRST (its DMA deps are ~224KB vs pair 0's
            # ~1.9MB, so the PE starts real work ~5us earlier while the big
            # transfers stream in).  That inverts the write order of outp
            # bank 3, so its start/stop/evac flags follow the actual
            # processing order: first writer = tile 14, last = tile 13.
            # (pair-7-first hoisting tried and reverted: real work started
            # ~5us earlier but its low PE duty never promoted the HAM clock
            # gate, so the whole first head ran at 1.2GHz -- net loss.)
            hoist = False
            if hoist:
                # zero-init outp bank 3 (zeros @ zeros matmul, no DMA deps):
                # pair 7 is processed first, so bank 3's writers arrive out
                # of order and must all accumulate onto a cleared bank.
                nc.tensor.matmul(outp[:, 1536:2048], lhsT=wsb[:, 0:65],
                                 rhs=wsb[:, 0:512], start=True, stop=False)

            def emit_pv(j, c, e, p, half, po, is_i16):
                # p[:, half, po:po+(e-c)] holds this k-tile's probs for
                # q in [c, e); split by outp PSUM bank.
                for b in range(c // 512, (e + 511) // 512):
                    g0, g1 = max(c, 512 * b), min(e, 512 * (b + 1))
                    rhs = p[:, half, po + g0 - c:po + g1 - c]
                    if is_i16:
                        rhs = rhs.bitcast(fp16)
                    if hoist and b == 3:
                        # bank 3 was zero-initialized by a dedicated clear
                        # matmul (start=True is bank-granular on HW), so
                        # every PV here accumulates; last writer = tile 13.
                        start = False
                        stop = (j == 13)
                        evac = (j == 13 and g1 == 2048)
                    else:
                        start = (j == 0)
                        stop = (j == min(4 * b + 3, NKT - 1))
                        evac = (j % 4 == 3 and b == (j - 3) // 4
                                and g1 == 512 * (b + 1))
                    nc.tensor.matmul(outp[:, g0:g1], lhsT=vp[:, j, :],
                                     rhs=rhs, start=start, stop=stop)
                    # bank b is fully accumulated once its last-processed
                    # tile has written up to the bank's end
                    if evac:
                        emit_evac(b)

            t_idx = 0

            def emit_softmax(i, c, w, stt):
                nonlocal t_idx
                # head-0 pair-0 is forced onto the schraud path against the
                # u8 early mask (halved critical DMA; STT is 1x regardless)
                early = (h == 0 and i == 0)
                if early or t_idx % SCHRAUD_MOD in SCHRAUD_IDX:
                    # full-DVE path in ONE op: i16 = (s*C + B) * mask
                    # (masked lanes exactly 0; bitcast fp16 = schraud exp)
                    msk = (mk8_sb[:, 0:2, c:c + w] if early
                           else mask_sb[:, 2 * i:2 * i + 2, c:c + w])
                    isch = s_pool.tile([P, 2, 512], i16, tag="isch")
                    nc.vector.scalar_tensor_tensor(
                        isch[:, 0:2, 0:w], stt[:, 0:2, 0:w], B_SCH,
                        msk, Add, Mult)
                    p, is16 = isch, True
                else:
                    p = p_pool.tile([P, 2, 512], fp16, tag="p")
                    nc.scalar.activation(p[:, 0:2, 0:w], stt[:, 0:2, 0:w],
                                         Exp, scale=ACT_SCALE)
                    # (GPSIMD offload of this mult re-tested with batched
                    # PVs: still +14us -- the GPSIMD op stalls the pipeline
                    # regardless of consumer slack.  Keep it on DVE.)
                    nc.vector.tensor_mul(p[:, 0:2, 0:w], p[:, 0:2, 0:w],
                                         mask_sb[:, 2 * i:2 * i + 2, c:c + w])
                    is16 = False
                t_idx += 1
                # even tile: full seg; odd tile: skip its below-diagonal
                # 128-col block on the first seg (masked anyway)
                pending_pv.append((2 * i, c, c + w, p, 0, 0, is16))
                if c == 256 * i:
                    pending_pv.append((2 * i + 1, c + P, c + w, p, 1, P, is16))
                else:
                    pending_pv.append((2 * i + 1, c, c + w, p, 1, 0, is16))
                # drain PVs in batches: each QK<->PV transition costs a PE
                # tiling-mode drain (~140ns), so batch same-kind matmuls.
                # Within a batch, sort by (tile, q) so consecutive PV MMs
                # share the same stationary vp weights (fewer LDWEIGHTS);
                # j-ascending order preserves the evac/flag invariants.
                # Near the end of the LAST head, drain eagerly to shorten
                # the serial tail after the final softmax.
                if h == HPC - 1 and i >= 6:
                    thresh, pop = 8, 4
                else:
                    thresh, pop = PV_DELAY + PV_POP, PV_POP
                if len(pending_pv) >= thresh:
                    batch = [pending_pv.pop(0) for _ in range(pop)]
                    batch.sort(key=lambda t: (t[0], t[1]))
                    for ent in batch:
                        emit_pv(*ent)

            order = [7, 0, 1, 2, 3, 4, 5, 6] if hoist else list(range(NPAIR))
            for n_i, i in enumerate(order):
                # prefetch the next head's inputs mid-head so their DMAs
                # finish well before the head boundary
                if n_i == 4 and h + 1 < HPC:
                    head_tiles[h + 1] = load_head(h + 1)
                # odd pairs start with a 256-wide seg so every later seg
                # (and thus every PV piece) is 512-aligned to outp banks
                c = 256 * i
                first = True
                while c < S:
                    w = min(256 if (first and i % 2) else 512, S - c)
                    first = False
                    stt = st_psum.tile([P, 2, 512], f32, tag="st")
                    nc.tensor.matmul(stt[:, 0, 0:w], lhsT=kt[0:64, i, :],
                                     rhs=qt[0:64, c:c + w],
                                     start=True, stop=True)
                    nc.tensor.matmul(stt[:, 1, 0:w], lhsT=kt[64:128, i, :],
                                     rhs=qt[64:128, c:c + w],
                                     start=True, stop=True)
                    emit_softmax(i, c, w, stt)
                    c += w

            pending_pv.sort(key=lambda t: (t[0], t[1]))
            while pending_pv:
                emit_pv(*pending_pv.pop(0))

    nc.compile()
    return nc


def prep_inputs(query, key, value, attn_mask):
    """Host-side layout prep (transposes/retiling/casts only) -> 8 in_maps."""
    query = np.asarray(query, dtype=np.float32)
    key = np.asarray(key, dtype=np.float32)
    value = np.asarray(value, dtype=np.float32)
    attn_mask = np.asarray(attn_mask).astype(bool)

    # Q pre-scaled by C_SCH so the QK matmul emits s*C directly; duplicated
    # on both partition halves for the two concurrent row-tiles.
    qT = (query.transpose(0, 1, 3, 2) * C_SCH).astype(np.float16)
    qT = np.ascontiguousarray(np.concatenate([qT, qT], axis=2))  # [B,H,128,S]

    # K^T packed in k-tile pairs: [B,H,128,NPAIR,128] with partitions 0-63 =
    # even k-tiles' K^T, 64-127 = odd k-tiles'.
    kT = key.transpose(0, 1, 3, 2).astype(np.float16)            # [B,H,64,S]
    kp = kT.reshape(B, H, D, NPAIR, 2, P)
    ktp = np.empty((B, H, P, NPAIR, P), np.float16)
    ktp[:, :, 0:D] = kp[:, :, :, :, 0, :]
    ktp[:, :, D:P] = kp[:, :, :, :, 1, :]

    vp = np.concatenate(
        [value, np.ones((B, H, S, 1), np.float32)], axis=3).astype(np.float16)
    # [B, H, S, 65] -> [B, H, 128, NKT, 65] (partition-contiguous tiles)
    vp = np.ascontiguousarray(
        vp.reshape(B, H, NKT, P, D + 1).transpose(0, 1, 3, 2, 4))

    tril = np.tril(np.ones((S, S), dtype=bool))
    in_maps = []
    for b in range(B):
        m = (attn_mask[b] & tril)          # [q, k]
        mT = m.T.astype(np.float16)        # [k, q]
        maskt = np.ascontiguousarray(
            mT.reshape(NKT, P, S).transpose(1, 0, 2))  # [128, NKT, S]
        mask8 = np.ascontiguousarray(
            (maskt[:, 0:2, :] != 0).astype(np.uint8))   # [128, 2, S]
        for cl in range(NCORES // B):
            h0 = cl * HPC
            in_maps.append({
                "qt": np.ascontiguousarray(qT[b, h0:h0 + HPC]),
                "kt": np.ascontiguousarray(ktp[b, h0:h0 + HPC]),
                "vp": np.ascontiguousarray(vp[b, h0:h0 + HPC]),
                "maskt": maskt,
                "mask8": mask8,
            })
    return in_maps


def run(query, key, value, attn_mask, trace=False, trace_cores=None):
    from concourse import bass_utils

    if "nc" not in _cache:
        _cache["nc"] = build_nc()
    nc = _cache["nc"]

    in_maps = prep_inputs(query, key, value, attn_mask)
    res = bass_utils.run_bass_kernel_spmd(
        nc, in_maps, core_ids=list(range(NCORES)),
        trace=trace, trace_cores=trace_cores)

    out = np.empty((B, H, S, D), np.float32)
    for c in range(NCORES):
        b = c // (NCORES // B)
        h0 = (c % (NCORES // B)) * HPC
        outt = res.results[c]["outt"].astype(np.float32)   # [HPC, 65, S]
        num = outt[:, 0:D, :]                              # [HPC, 64, S]
        den = outt[:, D:D + 1, :]                          # [HPC, 1, S]
        out[b, h0:h0 + HPC] = (num / den).transpose(0, 2, 1)
    return out, res


def kernel(query, key, value, attn_mask):
    out, _ = run(query, key, value, attn_mask)
    return out


# revision 48
# speedup vs baseline: 1.0170x; 1.0170x over previous
# Trainium2 Bass kernel for masked causal attention
#   B=2, H=16, S=2048, D=64, bool attn_mask [B, S, S] + causal, softmax, @V.
#
# Sharding: 8 cores x 4 heads (cores 0-3 -> batch 0, cores 4-7 -> batch 1).
# Each core computes its 4 heads fully on-device; the per-batch mask is
# resident in SBUF and shared by the core's 4 heads.
#
# QK runs ROW-TILED: contraction is d=64, so the 128x128 PE array is split
# into two 64x128 tiles (T0 rows 0-63, T8 rows 64-127) that compute TWO
# k-tiles (a "pair") CONCURRENTLY -- q^T is duplicated on both partition
# halves, K^T of even k-tiles lives on partitions 0-63 and odd k-tiles on
# 64-127.  This halves QK stream time vs the old zero-padded 128-row form.
# PV stays full 128x128 (contraction = 128 keys); QK<->PV transitions now
# cost a tiling-mode drain (~140ns), so PVs are drained in batches.
#
# Per (head, pair i of k-tiles (2i, 2i+1), 512-wide q-segment from 256*i):
#   S^T[k, q] even/odd = concurrent 64-row matmuls -> [128, 2, 512] PSUM
#   softmax tile (one op over both halves):
#     2/3 of segs: ACT exp(s * ln2/1024) then DVE mask-multiply
#     1/3 of segs: ONE DVE scalar_tensor_tensor: i16 = (s*C + B)*mask,
#                  p = bitcast_fp16(i16)   (Q pre-scaled by C on host)
#   outT[m,q] += sum_k vp[k,m] p[k,q]   (PE: lhsT=[V | ones] -> row 64 = denom)
# The normalization (num/den) happens ON HOST: the kernel ships the
# unnormalized [65, S] accumulator as fp16.
# Causal structure: pair i only computes q >= 256*i; the odd tile's
# below-diagonal 128-col block is masked in softmax and skipped in PV.

import os
import numpy as np

B, H, S, D = 2, 16, 2048, 64
NCORES = 8
HPC = 4          # heads per core
P = 128
NKT = S // P     # 16 k-tiles
NPAIR = NKT // 2
CHUNK = 1024     # st psum tile: [128, 2, 512] f32 = 2 PSUM banks
SCHRAUD_IDX = {1}
SCHRAUD_MOD = 3
PV_DELAY = int(os.environ.get("ATTN_PV_DELAY", "8"))
PV_POP = int(os.environ.get("ATTN_PV_POP", "10"))
# 14 warm-up MMs end ~15.7us; head-0's DMA-stalled QK resumes ~17.7us, so
# the PE idle gap stays under the ~3.4us HAM window and the body never
# drops back to 1.2GHz (at 11 the gap re-throttled the clock until ~24us).
WARM_MM = int(os.environ.get("ATTN_WARM_MM", "14"))

# Schraudolph constants for p = exp(s * 0.125) via fp16 bit pattern:
#   i16 = int16(s*C_S + B_S);  p = bitcast_fp16(i16);  c=44 centers the
# relative error of the linear-mantissa approximation (max ~3.1%, rms ~2%).
C_SCH = float(0.125 * 1024.0 / np.log(2.0))
B_SCH = float(15.0 * 1024.0 - 44.0)
ACT_SCALE = float(np.log(2.0) / 1024.0)   # undoes the host C_SCH prescale

_cache = {}


def build_nc():
    import concourse.bacc as bacc
    import concourse.mybir as mybir
    import concourse.tile as tile
    from contextlib import ExitStack

    fp16 = mybir.dt.float16
    f32 = mybir.dt.float32
    i16 = mybir.dt.int16
    Exp = mybir.ActivationFunctionType.Exp
    Copy = mybir.ActivationFunctionType.Copy
    Mult = mybir.AluOpType.mult
    Add = mybir.AluOpType.add

    nc = bacc.Bacc("TRN2", target_bir_lowering=False, debug=False,
                   num_devices=NCORES)

    # Host-prepared, per-core inputs.  qt: q^T * C_SCH duplicated on both
    # partition halves (rows 0-63 == rows 64-127).  kt: K^T packed in pairs,
    # partitions 0-63 = even k-tiles, 64-127 = odd k-tiles.
    qt_d = nc.dram_tensor("qt", [HPC, P, S], fp16, kind="ExternalInput")
    kt_d = nc.dram_tensor("kt", [HPC, P, NPAIR, P], fp16, kind="ExternalInput")
    vp_d = nc.dram_tensor("vp", [HPC, P, NKT, D + 1], fp16, kind="ExternalInput")
    mk_d = nc.dram_tensor("maskt", [P, NKT, S], fp16, kind="ExternalInput")
    # unnormalized output: rows 0..63 = numerator^T, row 64 = denominator
    out_d = nc.dram_tensor("outt", [HPC, D + 1, S], fp16, kind="ExternalOutput")

    with tile.TileContext(nc) as tc, ExitStack() as ctx:
        mask_pool = ctx.enter_context(tc.tile_pool(name="mask", bufs=1))
        qk_pool = ctx.enter_context(tc.tile_pool(name="qk", bufs=2))
        vp_pool = ctx.enter_context(tc.tile_pool(name="vpool", bufs=2))
        p_pool = ctx.enter_context(tc.tile_pool(name="p", bufs=11))
        s_pool = ctx.enter_context(tc.tile_pool(name="sch", bufs=6))
        o_pool = ctx.enter_context(tc.tile_pool(name="osb", bufs=4))
        warm_pool = ctx.enter_context(tc.tile_pool(name="warm", bufs=1))
        st_psum = ctx.enter_context(tc.tile_pool(name="st", bufs=2, space="PSUM"))
        o_psum = ctx.enter_context(tc.tile_pool(name="outp", bufs=1, space="PSUM"))

        # PE warm-up on zeros: runs while the first input DMAs land and
        # opens the HAM clock gate to 2.4 GHz before the first real QK.
        wsb = warm_pool.tile([P, 512], fp16, tag="warm")
        nc.vector.memset(wsb[:], 0.0)
        wps = st_psum.tile([P, 2, 512], f32, tag="st")
        for i in range(WARM_MM):
            nc.tensor.matmul(wps[:, i % 2, :], lhsT=wsb[:, 0:P],
                             rhs=wsb[:], start=True, stop=True)

        def load_head(h):
            qt = qk_pool.tile([P, S], fp16, tag="qt")
            kt = qk_pool.tile([P, NPAIR, P], fp16, tag="kt")
            vp = vp_pool.tile([P, NKT, D + 1], fp16, tag="vp")
            nc.sync.dma_start(qt[:], qt_d[h])
            nc.sync.dma_start(kt[:], kt_d[h])
            nc.sync.dma_start(vp[:], vp_d[h])
            return qt, kt, vp

        # Start-latency-ordered first loads: the pair-0 seg-0 QK unblocks on
        # kt pair 0 (32KB) + first 512 q's (128KB), then mask planes 0-1 for
        # the first softmax, then the rest of head 0; remaining mask planes
        # stream behind.  Causal trim: k-tile g's mask is only read for
        # q >= 128*(g - g%2), so skip the lower-triangle bytes.
        qt0 = qk_pool.tile([P, S], fp16, tag="qt")
        kt0 = qk_pool.tile([P, NPAIR, P], fp16, tag="kt")
        vp0 = vp_pool.tile([P, NKT, D + 1], fp16, tag="vp")
        mask_sb = mask_pool.tile([P, NKT, S], fp16, tag="mask")
        # first-use order with coarse kicks (each dma_start costs ~0.65us
        # of Sync issue time).  Head 0 processes pair 7 FIRST: its deps
        # (kt pair 7 + qt/mask tails, ~224KB) land ~5us before pair 0's
        # ~1.9MB set, so the PE starts real work early.
        nc.sync.dma_start(kt0[:, 0:1, :], kt_d[0, :, 0:1, :])
        nc.sync.dma_start(qt0[:, 0:512], qt_d[0, :, 0:512])
        nc.sync.dma_start(mask_sb[:, 0:2, 0:512], mk_d[:, 0:2, 0:512])
        nc.sync.dma_start(qt0[:, 512:CHUNK], qt_d[0, :, 512:CHUNK])
        nc.sync.dma_start(mask_sb[:, 0:2, 512:], mk_d[:, 0:2, 512:])
        nc.sync.dma_start(kt0[:, 1:, :], kt_d[0, :, 1:, :])
        nc.sync.dma_start(qt0[:, CHUNK:], qt_d[0, :, CHUNK:])
        nc.sync.dma_start(vp0[:], vp_d[0])
        head_tiles = {0: (qt0, kt0, vp0)}
        for g in range(2, NKT):
            c0 = P * (g - (g % 2))
            nc.sync.dma_start(mask_sb[:, g:g + 1, c0:], mk_d[:, g:g + 1, c0:])

        for h in range(HPC):
            qt, kt, vp = head_tiles.pop(h, None) or load_head(h)
            outp = o_psum.tile([D + 1, S], f32, tag="outp")
            pending_pv = []

            def emit_evac(b):
                # bank b of outp ([65, 512] f32) is fully accumulated ->
                # convert to fp16 in SBUF and ship; host divides num/den.
                s0, s1 = 512 * b, 512 * (b + 1)
                osb = o_pool.tile([D + 1, 512], fp16, tag="osb")
                # 2.5 evacs on ACT / 1.5 on DVE on average (alternating by
                # head): post-rebalance ACT ~64.3us vs DVE ~65.1us busy.
                # (Copy lives in the same ACT table as Exp: no table reload.)
                if b not in ((1,) if h % 2 == 0 else (1, 3)):
                    nc.scalar.activation(osb[:], outp[:, s0:s1], Copy)
                else:
                    nc.vector.tensor_copy(osb[:], outp[:, s0:s1])
                nc.sync.dma_start(out_d[h, :, s0:s1], osb[:])

            # Head 0 runs pair 7 FIRST (its DMA deps are ~224KB vs pair 0's
            # ~1.9MB, so the PE starts real work ~5us earlier while the big
            # transfers stream in).  That inverts the write order of outp
            # bank 3, so its start/stop/evac flags follow the actual
            # processing order: first writer = tile 14, last = tile 13.
            # (pair-7-first hoisting tried and reverted: real work started
            # ~5us earlier but its low PE duty never promoted the HAM clock
            # gate, so the whole first head ran at 1.2GHz -- net loss.)
            hoist = False
            if hoist:
                # zero-init outp bank 3 (zeros @ zeros matmul, no DMA deps):
                # pair 7 is processed first, so bank 3's writers arrive out
                # of order and must all accumulate onto a cleared bank.
                nc.tensor.matmul(outp[:, 1536:2048], lhsT=wsb[:, 0:65],
                                 rhs=wsb[:, 0:512], start=True, stop=False)

            def emit_pv(j, c, e, p, half, po, is_i16):
                # p[:, half, po:po+(e-c)] holds this k-tile's probs for
                # q in [c, e); split by outp PSUM bank.
                for b in range(c // 512, (e + 511) // 512):
                    g0, g1 = max(c, 512 * b), min(e, 512 * (b + 1))
                    rhs = p[:, half, po + g0 - c:po + g1 - c]
                    if is_i16:
                        rhs = rhs.bitcast(fp16)
                    if hoist and b == 3:
                        # bank 3 was zero-initialized by a dedicated clear
                        # matmul (start=True is bank-granular on HW), so
                        # every PV here accumulates; last writer = tile 13.
                        start = False
                        stop = (j == 13)
                        evac = (j == 13 and g1 == 2048)
                    else:
                        start = (j == 0)
                        stop = (j == min(4 * b + 3, NKT - 1))
                        evac = (j % 4 == 3 and b == (j - 3) // 4
                                and g1 == 512 * (b + 1))
                    nc.tensor.matmul(outp[:, g0:g1], lhsT=vp[:, j, :],
                                     rhs=rhs, start=start, stop=stop)
                    # bank b is fully accumulated once its last-processed
                    # tile has written up to the bank's end
                    if evac:
                        emit_evac(b)

            t_idx = 0

            def emit_softmax(i, c, w, stt):
                nonlocal t_idx
                if t_idx % SCHRAUD_MOD in SCHRAUD_IDX:
                    # full-DVE path in ONE op: i16 = (s*C + B) * mask
                    # (masked lanes exactly 0; bitcast fp16 = schraud exp)
                    isch = s_pool.tile([P, 2, 512], i16, tag="isch")
                    nc.vector.scalar_tensor_tensor(
                        isch[:, 0:2, 0:w], stt[:, 0:2, 0:w], B_SCH,
                        mask_sb[:, 2 * i:2 * i + 2, c:c + w], Add, Mult)
                    p, is16 = isch, True
                else:
                    p = p_pool.tile([P, 2, 512], fp16, tag="p")
                    nc.scalar.activation(p[:, 0:2, 0:w], stt[:, 0:2, 0:w],
                                         Exp, scale=ACT_SCALE)
                    # (GPSIMD offload of this mult re-tested with batched
                    # PVs: still +14us -- the GPSIMD op stalls the pipeline
                    # regardless of consumer slack.  Keep it on DVE.)
                    nc.vector.tensor_mul(p[:, 0:2, 0:w], p[:, 0:2, 0:w],
                                         mask_sb[:, 2 * i:2 * i + 2, c:c + w])
                    is16 = False
                t_idx += 1
                # even tile: full seg; odd tile: skip its below-diagonal
                # 128-col block on the first seg (masked anyway)
                pending_pv.append((2 * i, c, c + w, p, 0, 0, is16))
                if c == 256 * i:
                    pending_pv.append((2 * i + 1, c + P, c + w, p, 1, P, is16))
                else:
                    pending_pv.append((2 * i + 1, c, c + w, p, 1, 0, is16))
                # drain PVs in batches: each QK<->PV transition costs a PE
                # tiling-mode drain (~140ns), so batch same-kind matmuls.
                # Within a batch, sort by (tile, q) so consecutive PV MMs
                # share the same stationary vp weights (fewer LDWEIGHTS);
                # j-ascending order preserves the evac/flag invariants.
                # Near the end of the LAST head, drain eagerly to shorten
                # the serial tail after the final softmax.
                if h == HPC - 1 and i >= 6:
                    thresh, pop = 8, 4
                else:
                    thresh, pop = PV_DELAY + PV_POP, PV_POP
                if len(pending_pv) >= thresh:
                    batch = [pending_pv.pop(0) for _ in range(pop)]
                    batch.sort(key=lambda t: (t[0], t[1]))
                    for ent in batch:
                        emit_pv(*ent)

            order = [7, 0, 1, 2, 3, 4, 5, 6] if hoist else list(range(NPAIR))
            for n_i, i in enumerate(order):
                # prefetch the next head's inputs mid-head so their DMAs
                # finish well before the head boundary
                if n_i == 4 and h + 1 < HPC:
                    head_tiles[h + 1] = load_head(h + 1)
                # odd pairs start with a 256-wide seg so every later seg
                # (and thus every PV piece) is 512-aligned to outp banks
                c = 256 * i
                first = True
                while c < S:
                    w = min(256 if (first and i % 2) else 512, S - c)
                    first = False
                    stt = st_psum.tile([P, 2, 512], f32, tag="st")
                    nc.tensor.matmul(stt[:, 0, 0:w], lhsT=kt[0:64, i, :],
                                     rhs=qt[0:64, c:c + w],
                                     start=True, stop=True)
                    nc.tensor.matmul(stt[:, 1, 0:w], lhsT=kt[64:128, i, :],
                                     rhs=qt[64:128, c:c + w],
                                     start=True, stop=True)
                    emit_softmax(i, c, w, stt)
                    c += w

            pending_pv.sort(key=lambda t: (t[0], t[1]))
            while pending_pv:
                emit_pv(*pending_pv.pop(0))

    nc.compile()
    return nc


def prep_inputs(query, key, value, attn_mask):
    """Host-side layout prep (transposes/retiling/casts only) -> 8 in_maps."""
    query = np.asarray(query, dtype=np.float32)
    key = np.asarray(key, dtype=np.float32)
    value = np.asarray(value, dtype=np.float32)
    attn_mask = np.asarray(attn_mask).astype(bool)

    # Q pre-scaled by C_SCH so the QK matmul emits s*C directly; duplicated
    # on both partition halves for the two concurrent row-tiles.
    qT = (query.transpose(0, 1, 3, 2) * C_SCH).astype(np.float16)
    qT = np.ascontiguousarray(np.concatenate([qT, qT], axis=2))  # [B,H,128,S]

    # K^T packed in k-tile pairs: [B,H,128,NPAIR,128] with partitions 0-63 =
    # even k-tiles' K^T, 64-127 = odd k-tiles'.
    kT = key.transpose(0, 1, 3, 2).astype(np.float16)            # [B,H,64,S]
    kp = kT.reshape(B, H, D, NPAIR, 2, P)
    ktp = np.empty((B, H, P, NPAIR, P), np.float16)
    ktp[:, :, 0:D] = kp[:, :, :, :, 0, :]
    ktp[:, :, D:P] = kp[:, :, :, :, 1, :]

    vp = np.concatenate(
        [value, np.ones((B, H, S, 1), np.float32)], axis=3).astype(np.float16)
    # [B, H, S, 65] -> [B, H, 128, NKT, 65] (partition-contiguous tiles)
    vp = np.ascontiguousarray(
        vp.reshape(B, H, NKT, P, D + 1).transpose(0, 1, 3, 2, 4))

    tril = np.tril(np.ones((S, S), dtype=bool))
    in_maps = []
    for b in range(B):
        m = (attn_mask[b] & tril)          # [q, k]
        mT = m.T.astype(np.float16)        # [k, q]
        maskt = np.ascontiguousarray(
            mT.reshape(NKT, P, S).transpose(1, 0, 2))  # [128, NKT, S]
        for cl in range(NCORES // B):
            h0 = cl * HPC
            in_maps.append({
                "qt": np.ascontiguousarray(qT[b, h0:h0 + HPC]),
                "kt": np.ascontiguousarray(ktp[b, h0:h0 + HPC]),
                "vp": np.ascontiguousarray(vp[b, h0:h0 + HPC]),
                "maskt": maskt,
            })
    return in_maps


def run(query, key, value, attn_mask, trace=False, trace_cores=None):
    from concourse import bass_utils

    if "nc" not in _cache:
        _cache["nc"] = build_nc()
    nc = _cache["nc"]

    in_maps = prep_inputs(query, key, value, attn_mask)
    res = bass_utils.run_bass_kernel_spmd(
        nc, in_maps, core_ids=list(range(NCORES)),
        trace=trace, trace_cores=trace_cores)

    out = np.empty((B, H, S, D), np.float32)
    for c in range(NCORES):
        b = c // (NCORES // B)
        h0 = (c % (NCORES // B)) * HPC
        outt = res.results[c]["outt"].astype(np.float32)   # [HPC, 65, S]
        num = outt[:, 0:D, :]                              # [HPC, 64, S]
        den = outt[:, D:D + 1, :]                          # [HPC, 1, S]
        out[b, h0:h0 + HPC] = (num / den).transpose(0, 2, 1)
    return out, res


def kernel(query, key, value, attn_mask):
    out, _ = run(query, key, value, attn_mask)
    return out
